# revision 1
# baseline (speedup 1.0000x reference)
"""CharRNN Trainium2 kernel: data-parallel over batch on 8 NeuronCores.

kernel(**inputs) takes the FULL unsharded inputs (as produced by
setup_inputs) and returns the full [128, 1024, 128] float32 logits.
Each core runs 16 batch rows through the full T=1024 tanh recurrence
with 16-bit-precision (hi/lo bf16 split) matmuls on the TensorEngine.
"""
import numpy as np

"""CharRNN Bass kernel builder (per-core, data-parallel over batch).

Per core: B=16 batch rows, T steps, HID=512, VOCAB=128, EMB=16.

Layout: transposed state tiles hT [128 partitions = hidden-within-tile,
4 col-groups of 16 = (i-tile, batch)]. Recurrence matmuls keep W_hh
stationary (split hi/lo bf16 for 16-bit precision) and h moving
(split hi/lo bf16, 3-term product). xp is accumulated into the same
PSUM group via identity-stationary matmuls from precomputed bf16
hi/lo xp tiles. tanh runs twice on ACT (bf16 h_hi on the critical
chain, f32 for the h_lo residual, produced off-chain on DVE).

Phases interleaved per 32-step chunk:
  - xp phase for chunk c+1 (onehot build + M' matmuls + psum copies)
  - recurrence steps for chunk c
  - fc blocks every 8 steps for chunk c-1 (logits = hs @ W_fc.T + b_fc)
"""
import concourse.bacc as bacc
import concourse.mybir as mybir
from concourse.tile import TileContext
from concourse.masks import make_identity

f32 = mybir.dt.float32
bf16 = mybir.dt.bfloat16
i32 = mybir.dt.int32

B = 16        # batch rows per core
H = 512
NT = 4        # hidden tiles
V = 128
E = 16
CH = 32       # steps per xp chunk
FC = 8        # steps per fc block
AF = mybir.ActivationFunctionType
ALU = mybir.AluOpType


def build(T: int = 1024, do_rec=True, do_fc=True, do_xp=True, debug_out=False):
    assert T % CH == 0
    nc = bacc.Bacc("TRN2", target_bir_lowering=False, debug=False)

    # DRAM parameters (per-core shards / replicated weights)
    x_tb = nc.declare_dram_parameter("x_tb", [T, B], f32, isOutput=False)
    emb = nc.declare_dram_parameter("emb", [V, E], f32, isOutput=False)
    W_ih = nc.declare_dram_parameter("W_ih", [H, E], f32, isOutput=False)
    W_hh = nc.declare_dram_parameter("W_hh", [H, H], f32, isOutput=False)
    bias = nc.declare_dram_parameter("bias", [1, H], f32, isOutput=False)  # b_ih+b_hh
    W_fc = nc.declare_dram_parameter("W_fc", [V, H], f32, isOutput=False)
    b_fc = nc.declare_dram_parameter("b_fc", [1, V], f32, isOutput=False)
    out = nc.declare_dram_parameter("out", [B, T, V], f32, isOutput=True)
    if debug_out:
        dbg_mp = nc.declare_dram_parameter("dbg_mp", [128, H], f32, isOutput=True)
        dbg_xph = nc.declare_dram_parameter("dbg_xph", [128, CH * NT * B], f32, isOutput=True)
        dbg_xpl = nc.declare_dram_parameter("dbg_xpl", [128, CH * NT * B], f32, isOutput=True)
        dbg_hs = nc.declare_dram_parameter("dbg_hs", [3, 128, FC * NT * B], f32, isOutput=True)
        dbg_whh = nc.declare_dram_parameter("dbg_whh", [128, NT * H], f32, isOutput=True)
        dbg_embT = nc.declare_dram_parameter("dbg_embT", [E, V], f32, isOutput=True)
        dbg_wihT = nc.declare_dram_parameter("dbg_wihT", [E, H], f32, isOutput=True)
        dbg_mpps = nc.declare_dram_parameter("dbg_mpps", [128, H], f32, isOutput=True)
        dbg_onehot = nc.declare_dram_parameter("dbg_onehot", [128, CH * B], f32, isOutput=True)

    n_chunks = T // CH

    with TileContext(nc) as tc:
        with (
            tc.tile_pool(name="const", bufs=1) as cpool,
            tc.tile_pool(name="state", bufs=3) as spool,
            tc.tile_pool(name="hs", bufs=1) as hspool,
            tc.tile_pool(name="xp", bufs=1) as xppool,
            tc.tile_pool(name="work", bufs=2) as wkpool,
            tc.tile_pool(name="ps_rec", bufs=2, space="PSUM") as ps_rec,
            tc.tile_pool(name="ps_xp", bufs=2, space="PSUM") as ps_xp,
            tc.tile_pool(name="ps_fc", bufs=2, space="PSUM") as ps_fc,
        ):
            # ---------------- one-time prep ----------------
            ident_f32 = cpool.tile([128, 128], f32, tag="ident")
            make_identity(nc, ident_f32)
            ident_bf = cpool.tile([128, 128], bf16, tag="identb")
            nc.vector.tensor_copy(ident_bf[:, :], ident_f32[:, :])

            # W_hhT tiles [128_i, NT, 128_j] hi/lo bf16, via PE transpose of
            # natural-layout W_hh [j, i].
            w_nat = wkpool.tile([128, NT, H], f32, tag="wnat")  # [j_p, jt, i]
            nc.sync.dma_start(
                w_nat[:, :, :], W_hh.rearrange("(jt p) i -> p jt i", p=128)
            )
            whh_hi = cpool.tile([128, NT, H], bf16, tag="whh_hi")  # [i_p, it, j]
            whh_lo = cpool.tile([128, NT, H], bf16, tag="whh_lo")
            for it in range(NT):
                for jt in range(NT):
                    tp = ps_xp.tile([128, 128], f32, tag="xpp")
                    # in_ = W_hh block [j_p, i] for (jt, it); out = [i_p, j]
                    nc.tensor.transpose(
                        tp[:, :],
                        w_nat[:, jt, it * 128 : (it + 1) * 128],
                        ident_f32[:, :],
                    )
                    nc.vector.tensor_copy(
                        whh_hi[:, it, jt * 128 : (jt + 1) * 128], tp[:, :]
                    )
                    nc.vector.tensor_tensor(
                        whh_lo[:, it, jt * 128 : (jt + 1) * 128],
                        tp[:, :],
                        whh_hi[:, it, jt * 128 : (jt + 1) * 128],
                        ALU.subtract,
                    )

            # M' = emb @ W_ih.T + bias  -> [128_v, H], split hi/lo bf16
            embT = wkpool.tile([E, V], f32, tag="embT")
            nc.sync.dma_start(embT[:, :], emb.rearrange("v e -> e v"))
            wihT = wkpool.tile([E, H], f32, tag="wihT")
            nc.sync.dma_start(wihT[:, :], W_ih.rearrange("h e -> e h"))
            mp_ps = ps_xp.tile([128, H], f32, tag="xpp")
            nc.tensor.matmul(mp_ps[:, :], embT[:, :], wihT[:, :], start=True, stop=True)
            if debug_out:
                nc.sync.dma_start(dbg_embT[:, :], embT[:, :])
                nc.sync.dma_start(dbg_wihT[:, :], wihT[:, :])
                dmpps = wkpool.tile([128, H], f32, tag="dmpps")
                nc.vector.tensor_copy(dmpps[:, :], mp_ps[:, :])
                nc.sync.dma_start(dbg_mpps[:, :], dmpps[:, :])
            bias_row = wkpool.tile([1, H], f32, tag="biasrow")
            nc.sync.dma_start(bias_row[:, :], bias[:, :])
            bias_bc = wkpool.tile([128, H], f32, tag="biasbc")
            nc.gpsimd.partition_broadcast(bias_bc[:, :], bias_row[:, :])
            mprime = cpool.tile([128, H], f32, tag="mprime")
            nc.vector.tensor_tensor(mprime[:, :], mp_ps[:, :], bias_bc[:, :], ALU.add)
            mp_hi = cpool.tile([128, H], bf16, tag="mp_hi")
            mp_lo = cpool.tile([128, H], bf16, tag="mp_lo")
            nc.vector.tensor_copy(mp_hi[:, :], mprime[:, :])
            nc.vector.tensor_tensor(mp_lo[:, :], mprime[:, :], mp_hi[:, :], ALU.subtract)

            # W_fcT tiles [128_j, NT, 128_v] bf16 via PE transpose
            wfc_nat = wkpool.tile([128, H], f32, tag="wfcnat")  # [v_p, j]
            nc.sync.dma_start(wfc_nat[:, :], W_fc[:, :])
            wfcT = cpool.tile([128, NT, V], bf16, tag="wfcT")
            for jt in range(NT):
                tp = ps_xp.tile([128, 128], f32, tag="xpp")
                nc.tensor.transpose(
                    tp[:, :], wfc_nat[:, jt * 128 : (jt + 1) * 128], ident_f32[:, :]
                )
                nc.vector.tensor_copy(wfcT[:, jt, :], tp[:, :])

            # b_fc broadcast [128_tok, V]
            bfc_row = wkpool.tile([1, V], f32, tag="bfcrow")
            nc.sync.dma_start(bfc_row[:, :], b_fc[:, :])
            bfc_bc = cpool.tile([128, V], f32, tag="bfcbc")
            nc.gpsimd.partition_broadcast(bfc_bc[:, :], bfc_row[:, :])

            # iota column [128, 1] i32 for onehot compares
            iota_col = cpool.tile([128, 1], f32, tag="iota")
            nc.gpsimd.iota(iota_col[:, :], pattern=[[0, 1]], channel_multiplier=1,
                           allow_small_or_imprecise_dtypes=True)

            # initial h state (zeros)
            h_hi = spool.tile([128, NT * B], bf16, tag="h_hi_init")
            h_lo = spool.tile([128, NT * B], bf16, tag="h_lo_init")
            nc.vector.memset(h_hi[:, :], 0.0)
            nc.vector.memset(h_lo[:, :], 0.0)

            # xp chunk double buffers [128_j(4 groups? no: partitions=j-in-tile),
            # CH*NT*B cols: col = 64*s + 16*jt + b]
            xp_hi = [
                xppool.tile([128, CH * NT * B], bf16, tag=f"xp_hi{par}", name=f"xp_hi{par}")
                for par in range(2)
            ]
            xp_lo = [
                xppool.tile([128, CH * NT * B], bf16, tag=f"xp_lo{par}", name=f"xp_lo{par}")
                for par in range(2)
            ]
            # hs buffers per fc block: [128, FC*NT*B] bf16, col = 64*s + 16*it + b
            n_hs = 3
            hsbufs = [
                hspool.tile([128, FC * NT * B], bf16, tag=f"hs{k}", name=f"hs{k}") for k in range(n_hs)
            ]

            onehot_cur = [None]

            def xp_prep(c):
                """Build onehot for chunk c (off the PE chain)."""
                s0 = c * CH
                xrow = wkpool.tile([1, CH * B], f32, tag="xrow")
                nc.sync.dma_start(
                    xrow[:, :],
                    x_tb.rearrange("(a t) b -> a (t b)", t=CH)[c : c + 1, :],
                )
                xbc = wkpool.tile([128, CH * B], f32, tag="xbc")
                nc.gpsimd.partition_broadcast(xbc[:, :], xrow[:, :])
                onehot = wkpool.tile([128, CH * B], bf16, tag="onehot")
                nc.vector.tensor_scalar(
                    onehot[:, :], xbc[:, :], iota_col[:, :], None, ALU.is_equal
                )
                if debug_out and c == 0:
                    oh = wkpool.tile([128, CH * B], f32, tag="ohdbg")
                    nc.vector.tensor_copy(oh[:, :], onehot[:, :])
                    nc.sync.dma_start(dbg_onehot[:, :], oh[:, :])
                onehot_cur[0] = onehot

            xp_psum_cur = [None]
            xp_psum_pend = {}

            def xp_mm_one(c, jt, which):
                """One xp matmul (hi or lo) for j-tile jt of chunk c; after
                'lo', scatter psum into the per-step xp layout."""
                par = c % 2
                onehot = onehot_cur[0]
                if which == "hi":
                    ps = ps_xp.tile([128, CH * B], f32, tag="xpp")
                    xp_psum_cur[0] = ps
                    nc.tensor.matmul(
                        ps[:, :], mp_hi[:, jt * 128 : (jt + 1) * 128], onehot[:, :],
                        start=True, stop=False, skip_group_check=True,
                    )
                else:
                    ps = xp_psum_cur[0]
                    nc.tensor.matmul(
                        ps[:, :], mp_lo[:, jt * 128 : (jt + 1) * 128], onehot[:, :],
                        start=False, stop=True, skip_group_check=True,
                    )
                    xp_psum_pend[jt] = ps

            def xp_scatter(c, jt, piece):
                """Scatter 8 steps of xp psum for j-tile jt (small DVE ops)."""
                par = c % 2
                ps = xp_psum_pend[jt]
                q = CH // 4
                sl = slice(piece * q, (piece + 1) * q)
                dst_hi, dst_lo = xp_hi[par], xp_lo[par]
                dh = dst_hi.rearrange("p (s g b) -> p s g b", s=CH, g=NT)[:, sl, jt, :]
                dl = dst_lo.rearrange("p (s g b) -> p s g b", s=CH, g=NT)[:, sl, jt, :]
                sps = ps.rearrange("p (s b) -> p s b", s=CH)[:, sl, :]
                nc.vector.tensor_copy(dh, sps)
                nc.vector.tensor_tensor(dl, sps, dh, ALU.subtract)

            def xp_phase(c):
                xp_prep(c)
                for jt in range(NT):
                    xp_mm_one(c, jt, "hi")
                    xp_mm_one(c, jt, "lo")
                    for piece in range(4):
                        xp_scatter(c, jt, piece)

            fc_state = {}

            def fc_part(hsbuf, s0, phase):
                """logits for FC steps starting at s0 from hsbuf; phase 0
                repacks + first 2 matmuls, phase 1 finishes + stores."""
                if phase == -1:
                    fcbuf = wkpool.tile([128, NT * 128], bf16, tag="fcbuf")
                    fc_state[("buf", s0)] = fcbuf
                    nc.vector.tensor_copy(
                        fcbuf.rearrange("p (g b s) -> p s g b", g=NT, b=B)[:, :, 0:2, :],
                        hsbuf.rearrange("p (s g b) -> p s g b", s=FC, g=NT)[:, :, 0:2, :],
                    )
                    return
                if phase == 0:
                    fcbuf = fc_state.pop(("buf", s0))
                    nc.vector.tensor_copy(
                        fcbuf.rearrange("p (g b s) -> p s g b", g=NT, b=B)[:, :, 2:4, :],
                        hsbuf.rearrange("p (s g b) -> p s g b", s=FC, g=NT)[:, :, 2:4, :],
                    )
                    ps = ps_fc.tile([128, V], f32, tag="fcp")
                    fc_state[s0] = (fcbuf, ps)
                    for jt in (0, 1):
                        nc.tensor.matmul(
                            ps[:, :], fcbuf[:, jt * 128 : (jt + 1) * 128],
                            wfcT[:, jt, :],
                            start=(jt == 0), stop=False,
                            skip_group_check=(jt != 0),
                        )
                else:
                    fcbuf, ps = fc_state.pop(s0)
                    for jt in (2, 3):
                        nc.tensor.matmul(
                            ps[:, :], fcbuf[:, jt * 128 : (jt + 1) * 128],
                            wfcT[:, jt, :],
                            start=False, stop=(jt == 3),
                            skip_group_check=(jt != 3),
                        )
                    lg = wkpool.tile([128, V], f32, tag="logits")
                    nc.vector.tensor_tensor(lg[:, :], ps[:, :], bfc_bc[:, :], ALU.add)
                    # out[b, s0+s, v]: partitions = (b-major: p = b*FC + s)
                    nc.sync.dma_start(out[:, s0 : s0 + FC, :], lg[:, :])

            def fc_phase(hsbuf, s0):
                fc_part(hsbuf, s0, -1)
                fc_part(hsbuf, s0, 0)
                fc_part(hsbuf, s0, 1)

            def rec_step(c, s):
                """One recurrence step s (global), chunk c."""
                nonlocal h_hi, h_lo
                par = c % 2
                si = s - c * CH
                psum = ps_rec.tile([128, NT * B], f32, tag="rec")
                xh = xp_hi[par][:, si * 64 : (si + 1) * 64]
                xl = xp_lo[par][:, si * 64 : (si + 1) * 64]
                mms = [(psum[:, :], ident_bf[:, :], xh),
                       (psum[:, :], ident_bf[:, :], xl)]
                for jt in range(NT):
                    o = psum[:, jt * B : (jt + 1) * B]
                    for it in range(NT):
                        mms.append(
                            (o, whh_hi[:, it, jt * 128 : (jt + 1) * 128],
                             h_hi[:, it * B : (it + 1) * B])
                        )
                    for it in range(NT):
                        mms.append(
                            (o, whh_lo[:, it, jt * 128 : (jt + 1) * 128],
                             h_hi[:, it * B : (it + 1) * B])
                        )
                for jt in range(NT):
                    o = psum[:, jt * B : (jt + 1) * B]
                    for it in range(NT):
                        mms.append(
                            (o, whh_hi[:, it, jt * 128 : (jt + 1) * 128],
                             h_lo[:, it * B : (it + 1) * B])
                        )
                for k, (o, stat, mov) in enumerate(mms):
                    nc.tensor.matmul(
                        o, stat, mov,
                        start=(k == 0), stop=(k == len(mms) - 1),
                        skip_group_check=(k not in (0, len(mms) - 1)),
                    )
                # tanh -> h_hi (bf16, into hsbuf slice for fc) and f32 for lo
                hsbuf = hsbufs[(s // FC) % n_hs]
                new_hi = hsbuf[:, (s % FC) * 64 : (s % FC + 1) * 64]
                nc.scalar.activation(new_hi, psum[:, :], AF.Tanh)
                hT = spool.tile([128, NT * B], f32, tag="hT")
                nc.scalar.activation(hT[:, :], psum[:, :], AF.Tanh)
                new_lo = spool.tile([128, NT * B], bf16, tag="h_lo")
                nc.vector.tensor_tensor(new_lo[:, :], hT[:, :], new_hi, ALU.subtract)
                h_hi = new_hi
                h_lo = new_lo

            # ---------------- main schedule ----------------
            if do_xp:
                xp_phase(0)
            for c in range(n_chunks):
                for s in range(c * CH, (c + 1) * CH):
                    if do_rec:
                        rec_step(c, s)
                    si = s - c * CH
                    # one fill op per step inside the tanh wait window:
                    # xp matmuls for chunk c+1 at odd si in [1, 16)
                    if do_xp and c + 1 < n_chunks:
                        if si == 0:
                            xp_prep(c + 1)
                        elif si % 2 == 1 and si < 16:
                            k = si // 2  # 0..7
                            xp_mm_one(c + 1, k // 2, "hi" if k % 2 == 0 else "lo")
                        if 4 <= si < 20 and (si - 4) % 4 == 0:
                            pass
                        if 4 <= si < 20:
                            jt_s, piece = (si - 4) // 4, (si - 4) % 4
                            if si - 4 >= 4 * jt_s and jt_s * 2 + 1 <= (si - 1) // 2:
                                xp_scatter(c + 1, jt_s, piece)
                    # fc split across two steps, lagging FC*2 behind
                    if do_fc:
                        if (s + 1) % FC == 5 and s + 1 >= FC * 2 - 3:
                            blk = (s + 8) // FC - 2
                            fc_part(hsbufs[blk % n_hs], blk * FC, -1)
                        if (s + 1) % FC == 6 and s + 1 >= FC * 2 - 2:
                            blk = (s + 7) // FC - 2
                            fc_part(hsbufs[blk % n_hs], blk * FC, 0)
                        if (s + 1) % FC == 0 and s + 1 >= FC * 2:
                            blk = (s + 1) // FC - 2
                            fc_part(hsbufs[blk % n_hs], blk * FC, 1)
            # final two fc blocks
            if do_fc:
                for blk in (T // FC - 2, T // FC - 1):
                    fc_phase(hsbufs[blk % n_hs], blk * FC)

            if debug_out:
                dmp = wkpool.tile([128, H], f32, tag="dmp")
                nc.vector.tensor_copy(dmp[:, :], mprime[:, :])
                nc.sync.dma_start(dbg_mp[:, :], dmp[:, :])
                dxh = wkpool.tile([128, CH * NT * B], f32, tag="dxh")
                nc.vector.tensor_copy(dxh[:, :], xp_hi[0][:, :])
                nc.sync.dma_start(dbg_xph[:, :], dxh[:, :])
                dxl = wkpool.tile([128, CH * NT * B], f32, tag="dxl")
                nc.vector.tensor_copy(dxl[:, :], xp_lo[0][:, :])
                nc.sync.dma_start(dbg_xpl[:, :], dxl[:, :])
                for k in range(3):
                    dhs = wkpool.tile([128, FC * NT * B], f32, tag="dhs")
                    nc.vector.tensor_copy(dhs[:, :], hsbufs[k][:, :])
                    nc.sync.dma_start(dbg_hs[k, :, :], dhs[:, :])
                dwh = wkpool.tile([128, NT * H], f32, tag="dwh")
                nc.vector.tensor_copy(dwh[:, :], whh_hi[:, :, :].rearrange("p a b -> p (a b)"))
                nc.vector.tensor_tensor(dwh[:, :], dwh[:, :],
                    whh_lo[:, :, :].rearrange("p a b -> p (a b)"), ALU.add)
                nc.sync.dma_start(dbg_whh[:, :], dwh[:, :])

    nc.finalize()
    return nc


_NC_CACHE = {}


def kernel(x, emb, W_ih, W_hh, b_ih, b_hh, W_fc, b_fc):
    from concourse.bass_utils import run_bass_kernel_spmd

    T_full = 1024
    x = np.asarray(x)
    emb = np.asarray(emb, dtype=np.float32)
    W_ih = np.asarray(W_ih, dtype=np.float32)
    W_hh = np.asarray(W_hh, dtype=np.float32)
    b_ih = np.asarray(b_ih, dtype=np.float32)
    b_hh = np.asarray(b_hh, dtype=np.float32)
    W_fc = np.asarray(W_fc, dtype=np.float32)
    b_fc = np.asarray(b_fc, dtype=np.float32)

    if "nc" not in _NC_CACHE:
        _NC_CACHE["nc"] = build(T_full)
    nc = _NC_CACHE["nc"]

    bias = (b_ih + b_hh).reshape(1, H).astype(np.float32)
    in_maps = []
    for core in range(8):
        xs = x[core * B : (core + 1) * B, :]          # [16, 1024]
        in_maps.append(dict(
            x_tb=np.ascontiguousarray(xs.T).astype(np.float32),
            emb=emb, W_ih=W_ih, W_hh=W_hh, bias=bias,
            W_fc=W_fc, b_fc=b_fc.reshape(1, V),
        ))
    res = run_bass_kernel_spmd(nc, in_maps, core_ids=list(range(8)))
    return np.concatenate([r["out"] for r in res.results], axis=0)



# revision 2
# speedup vs baseline: 1.5012x; 1.5012x over previous
"""CharRNN Trainium2 kernel: data-parallel over batch on 8 NeuronCores.

kernel(**inputs) takes the FULL unsharded inputs (as produced by
setup_inputs) and returns the full [128, 1024, 128] float32 logits.
Each core runs 16 batch rows through the full T=1024 tanh recurrence.

Precision scheme: fp16 hi/lo splits (effective ~2^-21) with three
matmul terms per step: Whi*h_hi, Wlo*h_hi, Whi*h_lo. The two Whi
terms share one stationary load: moving operand packs [h_hi|h_lo]
(N=32) and the PSUM output AP is broadcast (stride-0) over the pair
so both halves accumulate into the same 16 columns.

Per-step PE stream (34 matmuls):
  [issued in prior step's shadow] 2 identity matmuls init PSUM with
  xp_hi+xp_lo; then 16 Wlo*h_hi (N=16) needing only TANH#1's h_hi;
  then 16 packed Whi*[h_hi|h_lo] (N=32). TANH#1 (fp16 h_hi) and
  TANH#2 (f32) + DVE subtract (h_lo) run on ACT/DVE. Fillers (xp
  chunk matmuls split to N=256, fc matmuls, next-step inits) are
  placed in the tanh wait window.
"""
import numpy as np

import concourse.bacc as bacc
import concourse.mybir as mybir
from concourse.tile import TileContext
from concourse.masks import make_identity

f32 = mybir.dt.float32
f16 = mybir.dt.float16

B = 16        # batch rows per core
H = 512
NT = 4        # hidden tiles
V = 128
E = 16
CH = 32       # steps per xp chunk
FC = 8        # steps per fc block
AF = mybir.ActivationFunctionType
ALU = mybir.AluOpType

PACK = True   # stride-0 packed Whi*[h_hi|h_lo] matmuls


def build(T: int = 1024):
    assert T % CH == 0
    nc = bacc.Bacc("TRN2", target_bir_lowering=False, debug=False)

    x_tb = nc.declare_dram_parameter("x_tb", [T, B], f32, isOutput=False)
    emb = nc.declare_dram_parameter("emb", [V, E], f32, isOutput=False)
    W_ih = nc.declare_dram_parameter("W_ih", [H, E], f32, isOutput=False)
    W_hh = nc.declare_dram_parameter("W_hh", [H, H], f32, isOutput=False)
    bias = nc.declare_dram_parameter("bias", [1, H], f32, isOutput=False)  # b_ih+b_hh
    W_fc = nc.declare_dram_parameter("W_fc", [V, H], f32, isOutput=False)
    b_fc = nc.declare_dram_parameter("b_fc", [1, V], f32, isOutput=False)
    out = nc.declare_dram_parameter("out", [B, T, V], f32, isOutput=True)

    n_chunks = T // CH

    with TileContext(nc) as tc:
        with (
            tc.tile_pool(name="const", bufs=1) as cpool,
            tc.tile_pool(name="state", bufs=3) as spool,
            tc.tile_pool(name="hs", bufs=1) as hspool,
            tc.tile_pool(name="xp", bufs=1) as xppool,
            tc.tile_pool(name="work", bufs=2) as wkpool,
            tc.tile_pool(name="ps_rec", bufs=2, space="PSUM") as ps_rec,
            tc.tile_pool(name="ps_xp", bufs=2, space="PSUM") as ps_xp,
            tc.tile_pool(name="ps_fc", bufs=2, space="PSUM") as ps_fc,
        ):
            # ---------------- one-time prep ----------------
            ident_f32 = cpool.tile([128, 128], f32, tag="ident")
            make_identity(nc, ident_f32)
            ident_h = cpool.tile([128, 128], f16, tag="identh")
            nc.vector.tensor_copy(ident_h[:, :], ident_f32[:, :])

            # W_hhT tiles [128_i, NT, 128_j] hi/lo fp16, via PE transpose of
            # natural-layout W_hh [j, i].
            w_nat = wkpool.tile([128, NT, H], f32, tag="wnat")  # [j_p, jt, i]
            nc.sync.dma_start(
                w_nat[:, :, :], W_hh.rearrange("(jt p) i -> p jt i", p=128)
            )
            whh_hi = cpool.tile([128, NT, H], f16, tag="whh_hi")  # [i_p, it, j]
            whh_lo = cpool.tile([128, NT, H], f16, tag="whh_lo")
            for it in range(NT):
                for jt in range(NT):
                    tp = ps_xp.tile([128, 256], f32, tag="xpp")
                    nc.tensor.transpose(
                        tp[:, 0:128],
                        w_nat[:, jt, it * 128 : (it + 1) * 128],
                        ident_f32[:, :],
                    )
                    nc.vector.tensor_copy(
                        whh_hi[:, it, jt * 128 : (jt + 1) * 128], tp[:, 0:128]
                    )
                    nc.vector.tensor_tensor(
                        whh_lo[:, it, jt * 128 : (jt + 1) * 128],
                        tp[:, 0:128],
                        whh_hi[:, it, jt * 128 : (jt + 1) * 128],
                        ALU.subtract,
                    )

            # M' = emb @ W_ih.T + bias  -> [128_v, H], split hi/lo fp16
            embT = wkpool.tile([E, V], f32, tag="embT")
            nc.sync.dma_start(embT[:, :], emb.rearrange("v e -> e v"))
            wihT = wkpool.tile([E, H], f32, tag="wihT")
            nc.sync.dma_start(wihT[:, :], W_ih.rearrange("h e -> e h"))
            mp_ps = ps_xp.tile([128, H], f32, tag="mpp")
            nc.tensor.matmul(mp_ps[:, :], embT[:, :], wihT[:, :], start=True, stop=True)
            bias_row = wkpool.tile([1, H], f32, tag="biasrow")
            nc.sync.dma_start(bias_row[:, :], bias[:, :])
            bias_bc = wkpool.tile([128, H], f32, tag="biasbc")
            nc.gpsimd.partition_broadcast(bias_bc[:, :], bias_row[:, :])
            mprime = wkpool.tile([128, H], f32, tag="mprime")
            nc.vector.tensor_tensor(mprime[:, :], mp_ps[:, :], bias_bc[:, :], ALU.add)
            mp_hi = cpool.tile([128, H], f16, tag="mp_hi")
            mp_lo = cpool.tile([128, H], f16, tag="mp_lo")
            nc.vector.tensor_copy(mp_hi[:, :], mprime[:, :])
            nc.vector.tensor_tensor(mp_lo[:, :], mprime[:, :], mp_hi[:, :], ALU.subtract)

            # W_fcT tiles [128_j, NT, 128_v] fp16 via PE transpose
            wfc_nat = wkpool.tile([128, H], f32, tag="wfcnat")  # [v_p, j]
            nc.sync.dma_start(wfc_nat[:, :], W_fc[:, :])
            wfcT = cpool.tile([128, NT, V], f16, tag="wfcT")
            for jt in range(NT):
                tp = ps_xp.tile([128, 256], f32, tag="xpp")
                nc.tensor.transpose(
                    tp[:, 0:128], wfc_nat[:, jt * 128 : (jt + 1) * 128], ident_f32[:, :]
                )
                nc.vector.tensor_copy(wfcT[:, jt, :], tp[:, 0:128])

            # b_fc broadcast [128_tok, V]
            bfc_row = wkpool.tile([1, V], f32, tag="bfcrow")
            nc.sync.dma_start(bfc_row[:, :], b_fc[:, :])
            bfc_bc = cpool.tile([128, V], f32, tag="bfcbc")
            nc.gpsimd.partition_broadcast(bfc_bc[:, :], bfc_row[:, :])

            # iota column [128, 1] for onehot compares
            iota_col = cpool.tile([128, 1], f32, tag="iota")
            nc.gpsimd.iota(iota_col[:, :], pattern=[[0, 1]], channel_multiplier=1,
                           allow_small_or_imprecise_dtypes=True)

            # xp chunk double buffers: col = 64*s + 16*g + b (g = hidden tile)
            xp_hi = [
                xppool.tile([128, CH * 64], f16, tag=f"xp_hi{p}", name=f"xp_hi{p}")
                for p in range(2)
            ]
            xp_lo = [
                xppool.tile([128, CH * 64], f16, tag=f"xp_lo{p}", name=f"xp_lo{p}")
                for p in range(2)
            ]
            # h/hs buffers: per step-slot 128 cols = [h_hi(4g x 16b) | h_lo(...)]
            n_hs = 3
            hsbufs = [
                hspool.tile([128, FC * 128], f16, tag=f"hs{k}", name=f"hs{k}")
                for k in range(n_hs)
            ]
            # zero-init the slot step -1 reads (buf 2, slot 7)
            nc.vector.memset(hsbufs[2][:, 7 * 128 : 8 * 128], 0.0)

            onehots = {}

            def onehot_build(c):
                """Build onehot for chunk c (off the PE chain)."""
                xrow = wkpool.tile([1, CH * B], f32, tag="xrow")
                nc.sync.dma_start(
                    xrow[:, :],
                    x_tb.rearrange("(a t) b -> a (t b)", t=CH)[c : c + 1, :],
                )
                xbc = wkpool.tile([128, CH * B], f32, tag="xbc")
                nc.gpsimd.partition_broadcast(xbc[:, :], xrow[:, :])
                onehot = wkpool.tile([128, CH * B], f16, tag="onehot")
                nc.vector.tensor_scalar(
                    onehot[:, :], xbc[:, :], iota_col[:, :], None, ALU.is_equal
                )
                onehots[c] = onehot

            # xp phase: 8 psum groups per chunk, group g=(jt, half):
            #   mm mp_hi[jt] x onehot[half cols] (N=256, start)
            #   mm mp_lo[jt] x onehot[half]      (N=256, stop)
            # then DVE scatter: psum -> xp_hi/xp_lo slices.
            xp_groups = {}

            def xp_mm(c, g, which):
                jt, hf = g // 2, g % 2
                onehot = onehots[c]
                mov = onehot[:, hf * 256 : (hf + 1) * 256]
                if which == "hi":
                    ps = ps_xp.tile([128, 256], f32, tag="xpp")
                    xp_groups[(c, g)] = ps
                    nc.tensor.matmul(
                        ps[:, :], mp_hi[:, jt * 128 : (jt + 1) * 128], mov,
                        start=True, stop=False, skip_group_check=True,
                    )
                else:
                    ps = xp_groups[(c, g)]
                    nc.tensor.matmul(
                        ps[:, :], mp_lo[:, jt * 128 : (jt + 1) * 128], mov,
                        start=False, stop=True, skip_group_check=True,
                    )

            def xp_scatter(c, g):
                """Scatter psum group g: 16 steps x 16b -> xp layout."""
                jt, hf = g // 2, g % 2
                ps = xp_groups.pop((c, g))
                par = c % 2
                # dst col = 64*s + 16*jt + b for s in [hf*16, hf*16+16)
                dh = xp_hi[par].rearrange("p (s g b) -> p s g b", s=CH, g=NT)[
                    :, hf * 16 : (hf + 1) * 16, jt, :]
                dl = xp_lo[par].rearrange("p (s g b) -> p s g b", s=CH, g=NT)[
                    :, hf * 16 : (hf + 1) * 16, jt, :]
                sps = ps.rearrange("p (s b) -> p s b", s=16)
                nc.vector.tensor_copy(dh, sps)
                nc.vector.tensor_tensor(dl, sps, dh, ALU.subtract)

            # ---- fc ----
            fc_state = {}

            def fc_part(blk, phase):
                """logits for fc block blk (steps blk*FC..+FC)."""
                hsbuf = hsbufs[blk % n_hs]
                s0 = blk * FC
                # h_hi cols of slot: [p, slot, 0:64]
                hh = hsbuf.rearrange("p (t g x) -> p t g x", t=FC, g=2)[:, :, 0, :]
                if phase == -1:
                    fcbuf = wkpool.tile([128, NT * 128], f16, tag="fcbuf")
                    fc_state[("buf", s0)] = fcbuf
                    nc.vector.tensor_copy(
                        fcbuf.rearrange("p (g b s) -> p s g b", g=NT, b=B)[:, :, 0:2, :],
                        hh.rearrange("p s (g b) -> p s g b", g=NT)[:, :, 0:2, :],
                    )
                    return
                if phase == 0:
                    fcbuf = fc_state.pop(("buf", s0))
                    nc.vector.tensor_copy(
                        fcbuf.rearrange("p (g b s) -> p s g b", g=NT, b=B)[:, :, 2:4, :],
                        hh.rearrange("p s (g b) -> p s g b", g=NT)[:, :, 2:4, :],
                    )
                    ps = ps_fc.tile([128, V], f32, tag="fcp")
                    fc_state[s0] = (fcbuf, ps)
                    for jt in (0, 1):
                        nc.tensor.matmul(
                            ps[:, :], fcbuf[:, jt * 128 : (jt + 1) * 128],
                            wfcT[:, jt, :],
                            start=(jt == 0), stop=False,
                            skip_group_check=(jt != 0),
                        )
                else:
                    fcbuf, ps = fc_state.pop(s0)
                    for jt in (2, 3):
                        nc.tensor.matmul(
                            ps[:, :], fcbuf[:, jt * 128 : (jt + 1) * 128],
                            wfcT[:, jt, :],
                            start=False, stop=(jt == 3),
                            skip_group_check=(jt != 3),
                        )
                    lg = wkpool.tile([128, V], f32, tag="logits")
                    nc.vector.tensor_tensor(lg[:, :], ps[:, :], bfc_bc[:, :], ALU.add)
                    nc.sync.dma_start(out[:, s0 : s0 + FC, :], lg[:, :])

            # ---- recurrence ----
            rec_psums = {}

            def rec_init(s):
                """PSUM init for step s: identity x (xp_hi, xp_lo). Issued in
                step s-1's shadow; no h dependency."""
                c, si = s // CH, s % CH
                par = c % 2
                psum = ps_rec.tile([128, 64], f32, tag="rec")
                rec_psums[s] = psum
                xh = xp_hi[par][:, si * 64 : (si + 1) * 64]
                xl = xp_lo[par][:, si * 64 : (si + 1) * 64]
                nc.tensor.matmul(psum[:, :], ident_h[:, :], xh,
                                 start=True, stop=False)
                nc.tensor.matmul(psum[:, :], ident_h[:, :], xl,
                                 start=False, stop=False, skip_group_check=True)

            def rec_mms(s):
                """The 32 h-dependent matmuls of step s (h = h_{s-1})."""
                psum = rec_psums[s]
                pb, pslot = ((s - 1) // FC) % n_hs, (s - 1) % FC
                hsb = hsbufs[pb]
                hh = hsb[:, pslot * 128 : pslot * 128 + 64]
                h2 = hsb.rearrange("p (t g x) -> p t g x", t=FC, g=2)[:, pslot, :, :]
                # 16 Wlo * h_hi (N=16) — only need TANH#1 output
                for it in range(NT):
                    for jt in range(NT):
                        nc.tensor.matmul(
                            psum[:, jt * B : (jt + 1) * B],
                            whh_lo[:, it, jt * 128 : (jt + 1) * 128],
                            hh[:, it * B : (it + 1) * B],
                            start=False, stop=False, skip_group_check=True,
                        )
                # 16 packed Whi * [h_hi|h_lo] (N=32, stride-0 out)
                for it in range(NT):
                    for jt in range(NT):
                        last = it == NT - 1 and jt == NT - 1
                        if PACK:
                            o = (psum[:, jt * B : (jt + 1) * B]
                                 .unsqueeze(1).broadcast_to([128, 2, B]))
                            mov = h2[:, :, it * B : (it + 1) * B]
                            nc.tensor.matmul(
                                o, whh_hi[:, it, jt * 128 : (jt + 1) * 128], mov,
                                start=False, stop=last, skip_group_check=True,
                            )
                        else:
                            for g in range(2):
                                nc.tensor.matmul(
                                    psum[:, jt * B : (jt + 1) * B],
                                    whh_hi[:, it, jt * 128 : (jt + 1) * 128],
                                    h2[:, g, it * B : (it + 1) * B],
                                    start=False, stop=(last and g == 1),
                                    skip_group_check=True,
                                )

            def rec_tail(s):
                """TANH#1 (h_hi f16), TANH#2 (f32), DVE subtract (h_lo)."""
                psum = rec_psums.pop(s)
                buf, slot = (s // FC) % n_hs, s % FC
                hsb = hsbufs[buf]
                hi = hsb[:, slot * 128 : slot * 128 + 64]
                lo = hsb[:, slot * 128 + 64 : slot * 128 + 128]
                nc.scalar.activation(hi, psum[:, :], AF.Tanh)
                hT = spool.tile([128, 64], f32, tag="hT")
                nc.scalar.activation(hT[:, :], psum[:, :], AF.Tanh)
                nc.vector.tensor_tensor(lo, hT[:, :], hi, ALU.subtract)

            # ---------------- main schedule ----------------
            # prologue: onehot + xp for chunk 0, onehot for chunk 1
            onehot_build(0)
            for g in range(8):
                xp_mm(0, g, "hi")
                xp_mm(0, g, "lo")
                xp_scatter(0, g)
            onehot_build(1)
            rec_init(0)

            for s in range(T):
                c, si = s // CH, s % CH
                rec_mms(s)
                rec_tail(s)
                # ---- fillers: execute during the s -> s+1 tanh window ----
                if s + 1 < T:
                    rec_init(s + 1)
                # xp matmuls for chunk c+1: one N=256 mm per step window,
                # pattern over si in [0, 16): even si = hi, odd si = lo(+scatter)
                if c + 1 < n_chunks and si < 16:
                    g = si // 2
                    if si % 2 == 0:
                        xp_mm(c + 1, g, "hi")
                    else:
                        xp_mm(c + 1, g, "lo")
                        xp_scatter(c + 1, g)
                # onehot for chunk c+2 built late in chunk c
                if c + 2 < n_chunks and si == 28:
                    onehot_build(c + 2)
                # fc split across steps, lagging 2 blocks
                if (s + 1) % FC == 5 and s + 1 >= FC * 2 - 3:
                    fc_part((s + 8) // FC - 2, -1)
                if (s + 1) % FC == 6 and s + 1 >= FC * 2 - 2:
                    fc_part((s + 7) // FC - 2, 0)
                if (s + 1) % FC == 0 and s + 1 >= FC * 2:
                    fc_part((s + 1) // FC - 2, 1)
            # final two fc blocks
            for blk in (T // FC - 2, T // FC - 1):
                fc_part(blk, -1)
                fc_part(blk, 0)
                fc_part(blk, 1)

    nc.finalize()
    return nc


_NC_CACHE = {}


def kernel(x, emb, W_ih, W_hh, b_ih, b_hh, W_fc, b_fc):
    from concourse.bass_utils import run_bass_kernel_spmd

    T_full = 1024
    x = np.asarray(x)
    emb = np.asarray(emb, dtype=np.float32)
    W_ih = np.asarray(W_ih, dtype=np.float32)
    W_hh = np.asarray(W_hh, dtype=np.float32)
    b_ih = np.asarray(b_ih, dtype=np.float32)
    b_hh = np.asarray(b_hh, dtype=np.float32)
    W_fc = np.asarray(W_fc, dtype=np.float32)
    b_fc = np.asarray(b_fc, dtype=np.float32)

    if "nc" not in _NC_CACHE:
        _NC_CACHE["nc"] = build(T_full)
    nc = _NC_CACHE["nc"]

    bias = (b_ih + b_hh).reshape(1, H).astype(np.float32)
    in_maps = []
    for core in range(8):
        xs = x[core * B : (core + 1) * B, :]          # [16, 1024]
        in_maps.append(dict(
            x_tb=np.ascontiguousarray(xs.T).astype(np.float32),
            emb=emb, W_ih=W_ih, W_hh=W_hh, bias=bias,
            W_fc=W_fc, b_fc=b_fc.reshape(1, V),
        ))
    res = run_bass_kernel_spmd(nc, in_maps, core_ids=list(range(8)))
    return np.concatenate([r["out"] for r in res.results], axis=0)
